# revision 6
# baseline (speedup 1.0000x reference)
"""AttnBlock (BatchNorm + single-head self-attention + residual) on 8 TRN2
NeuronCores, data-parallel over the batch dim (B=8, one batch per core).

Key optimizations over the straightforward version:
  * BatchNorm folded into the projection weights: q = (Wq.diag(s)) @ x + (Wq t
    + bq), so the normalized activation h is never materialized and nothing
    but the 2KB stats AllGather sits between the x DMA and the first matmul.
  * All matmuls run in fp8(e4m3) DoubleRow perf mode: both 128-row halves of
    the contraction are packed per PE pass (contraction 256/pass at 0.5
    cycles/row). Weights are pre-scaled by 16 to clear the fp8 subnormal
    range; each epilogue multiplies by 1/16 while adding the folded bias.
  * Attention is streamed over 4 query chunks of 512; scores for an m-tile
    pair land in one 2-bank PSUM tile, one exp per pair (PSUM->SBUF fp8, with
    a -2 shift to keep exp() inside fp8 range; the shift cancels in the
    softmax normalization), attn@V and the ones-matmul denominator accumulate
    in PSUM over all 16 m-tiles.
  * Softmax normalization is applied after attn@V (denominator is constant
    over the contraction dim of the output projection), and the bv/bp biases
    collapse into a single per-channel constant added to the residual.
  * Epilogue work is split between DVE (q/k bias-cast, 1/S, hA*1/S) and
    GpSimd/Pool (v cast, output epilogue) so the softmax exp on the scalar
    engine stays the only near-critical non-matmul stream.
"""
import sys

sys.path.insert(0, "/opt/trn_rl_repo")

import numpy as np
import concourse.bass as bass
from concourse import bacc
import concourse.tile as tile
from concourse import mybir
from concourse.bass_utils import run_bass_kernel_spmd

F32 = mybir.dt.float32
BF16 = mybir.dt.bfloat16
FP8 = mybir.dt.float8e4
AF = mybir.ActivationFunctionType
ALU = mybir.AluOpType
AX = mybir.AxisListType
DR = mybir.MatmulPerfMode.DoubleRow

P = 128
C = 256
N = 2048
B = 8
CT = C // P          # 2 channel tiles
NT = N // P          # 16 key-position tiles
MP = NT // 2         # 8 key-tile pairs (DoubleRow packs 2 per pass)
FD = 512             # query-chunk width (one PSUM bank of f32)
NCH = N // FD        # 4 query chunks
BN_EPS = 1e-5
SM_SCALE = float(C) ** -0.5
EXP_SHIFT = -2.0     # exp(score + shift) keeps e below fp8 max; cancels in S
WS = 16.0            # weight pre-scale (clears fp8 subnormals)
WSI = 1.0 / WS


def build():
    nc = bacc.Bacc(num_devices=B)
    x_ext = nc.declare_dram_parameter("x", [C, N], F32, isOutput=False)
    wq_ext = nc.declare_dram_parameter("wqt", [C, C], F32, isOutput=False)
    wk_ext = nc.declare_dram_parameter("wkt", [C, C], F32, isOutput=False)
    wv_ext = nc.declare_dram_parameter("wvt", [C, C], F32, isOutput=False)
    wp_ext = nc.declare_dram_parameter("wpt", [C, C], F32, isOutput=False)
    vec_ext = nc.declare_dram_parameter("vecs", [6, C], F32, isOutput=False)
    out_ext = nc.declare_dram_parameter("out", [C, N], F32, isOutput=True)

    cc_in = nc.dram_tensor("cc_in", [P, 4], F32)
    cc_out = nc.dram_tensor("cc_out", [P * B, 4], F32, addr_space="Shared")

    with tile.TileContext(nc) as tc:
        with (
            tc.tile_pool(name="persist", bufs=1) as pp,
            tc.tile_pool(name="epool", bufs=3) as ep,
            tc.tile_pool(name="a8pool", bufs=2) as ap8,
            tc.tile_pool(name="scratch", bufs=2) as scr,
        ):
            # ---------------------------------------------------------- loads
            x_sb = [pp.tile([P, N], F32, name=f"x{t}", tag=f"x{t}") for t in range(CT)]
            for t in range(CT):
                for g in range(4):
                    nc.sync.dma_start(
                        out=x_sb[t][:, g * 512 : (g + 1) * 512],
                        in_=x_ext[t * P : (t + 1) * P, g * 512 : (g + 1) * 512],
                    )

            # vecs: gamma(0) beta(1) bq(2) bk(3) bv(4) bp(5)
            vec_sb = pp.tile([P, 6, CT], F32, name="vec", tag="vec")
            nc.sync.dma_start(
                out=vec_sb[:], in_=vec_ext.ap().rearrange("v (t q) -> q v t", q=P)
            )

            wst = {}
            for name, ext in (
                ("wq", wq_ext), ("wk", wk_ext), ("wv", wv_ext), ("wp", wp_ext)
            ):
                st = pp.tile([P, CT, C], F32, name=f"{name}st", tag=f"{name}st")
                nc.sync.dma_start(
                    out=st[:], in_=ext.ap().rearrange("(kt p) o -> p kt o", p=P)
                )
                wst[name] = st

            # ---------------------------------------------------- local stats
            stats_sb = pp.tile([P, 2, CT], F32, name="stats", tag="stats")
            for t in range(CT):
                st6 = pp.tile([P, 4, 6], F32, name=f"st6_{t}", tag=f"st6_{t}")
                for g in range(4):
                    nc.vector.bn_stats(
                        out=st6[:, g, :], in_=x_sb[t][:, g * 512 : (g + 1) * 512]
                    )
                nc.vector.bn_aggr(out=stats_sb[:, :, t], in_=st6[:])

            nc.sync.dma_start(
                out=cc_in[:, :], in_=stats_sb.rearrange("p a b -> p (a b)")
            )
            nc.gpsimd.collective_compute(
                "AllGather",
                ALU.bypass,
                replica_groups=[list(range(B))],
                ins=[cc_in[:, :]],
                outs=[cc_out[:, :]],
            )

            # ------------------------- collective-shadow work (needs no stats)
            x8 = pp.tile([P, CT, N], FP8, name="x8", tag="x8")
            for t in range(CT):
                nc.scalar.copy(out=x8[:, t, :], in_=x_sb[t][:])
            wp8 = pp.tile([P, CT, C], FP8, name="wp8", tag="wp8")
            nc.gpsimd.tensor_scalar_mul(out=wp8[:], in0=wst["wp"][:], scalar1=WS)
            ones8 = pp.tile([P, 2, P], FP8, name="ones8", tag="ones8")
            nc.gpsimd.memset(ones8[:], 1.0)
            shift_ap = pp.tile([P, 1], F32, name="shift", tag="shift")
            nc.gpsimd.memset(shift_ap[:], EXP_SHIFT)

            # ------------------------------------------------- global stats
            # f index = stat*CT + t: f0=mean_t0 f1=mean_t1 f2=var_t0 f3=var_t1
            ag_sb = pp.tile([P, 4, B], F32, name="ag", tag="ag")
            nc.sync.dma_start(
                out=ag_sb[:], in_=cc_out.ap().rearrange("(r p) f -> p f r", p=P)
            )
            tot = pp.tile([P, 4], F32, name="tot", tag="tot")
            nc.vector.tensor_reduce(out=tot[:], in_=ag_sb[:], axis=AX.X, op=ALU.add)
            nc.vector.tensor_scalar_mul(tot[:], tot[:], 1.0 / B)
            msq8 = pp.tile([P, CT, B], F32, name="msq8", tag="msq8")
            nc.vector.tensor_tensor(
                msq8[:], ag_sb[:, 0:CT, :], ag_sb[:, 0:CT, :], op=ALU.mult
            )
            m2avg = pp.tile([P, CT], F32, name="m2avg", tag="m2avg")
            nc.vector.tensor_reduce(out=m2avg[:], in_=msq8[:], axis=AX.X, op=ALU.add)
            nc.vector.tensor_scalar_mul(m2avg[:], m2avg[:], 1.0 / B)
            # var_g = avg_var + avg(mean^2) - mean_g^2
            var_g = pp.tile([P, CT], F32, name="varg", tag="varg")
            nc.vector.tensor_tensor(var_g[:], tot[:, CT : 2 * CT], m2avg[:], op=ALU.add)
            msq = pp.tile([P, CT], F32, name="msq", tag="msq")
            nc.vector.tensor_tensor(msq[:], tot[:, 0:CT], tot[:, 0:CT], op=ALU.mult)
            nc.vector.tensor_tensor(var_g[:], var_g[:], msq[:], op=ALU.subtract)
            # rstd = exp(-0.5 * ln(var + eps))
            eps_ap = pp.tile([P, 1], F32, name="eps", tag="eps")
            nc.vector.memset(eps_ap[:], BN_EPS)
            lnv = pp.tile([P, CT], F32, name="lnv", tag="lnv")
            nc.scalar.activation(out=lnv[:], in_=var_g[:], func=AF.Ln, bias=eps_ap[:])
            rstd = pp.tile([P, CT], F32, name="rstd", tag="rstd")
            nc.scalar.activation(out=rstd[:], in_=lnv[:], func=AF.Exp, scale=-0.5)
            # s = gamma * rstd ; tvec = beta - mean_g * s ; s16 = WS * s
            s_sb = pp.tile([P, CT], F32, name="ssb", tag="ssb")
            nc.vector.tensor_tensor(s_sb[:], vec_sb[:, 0, :], rstd[:], op=ALU.mult)
            tvec = pp.tile([P, CT], F32, name="tvec", tag="tvec")
            nc.vector.tensor_tensor(tvec[:], tot[:, 0:CT], s_sb[:], op=ALU.mult)
            nc.vector.tensor_tensor(tvec[:], vec_sb[:, 1, :], tvec[:], op=ALU.subtract)
            s16 = pp.tile([P, CT], F32, name="s16", tag="s16")
            nc.vector.tensor_scalar_mul(s16[:], s_sb[:], WS)

            # ------------------------------------- fold BN scale into weights
            w8 = {}
            for name, eng in (("wk", nc.vector), ("wv", nc.gpsimd), ("wq", nc.vector)):
                w = pp.tile([P, CT, C], FP8, name=f"{name}8", tag=f"{name}8")
                for kt in range(CT):
                    eng.tensor_scalar_mul(
                        out=w[:, kt, :], in0=wst[name][:, kt, :],
                        scalar1=s16[:, kt : kt + 1],
                    )
                w8[name] = w
            w8["wp"] = wp8

            # ------------------------------------------- folded bias constants
            # cq = Wq t + bq, ck = Wk t + bk, cv = Wv t + bv,
            # cout = Wp cv + bp  (residual absorbs cout; bv never touches v)
            cq = pp.tile([P, CT], F32, name="cq", tag="cq")
            ck = pp.tile([P, CT], F32, name="ck", tag="ck")
            cv = pp.tile([P, CT], F32, name="cv", tag="cv")
            cout = pp.tile([P, CT], F32, name="cout", tag="cout")
            with tc.tile_pool(name="psum_cst", bufs=2, space="PSUM") as pc:
                for wname, dst, bidx in (("wq", cq, 2), ("wk", ck, 3), ("wv", cv, 4)):
                    for i in range(CT):
                        ps = pc.tile([P, 1], F32, name="cst", tag="cst")
                        for kt in range(CT):
                            nc.tensor.matmul(
                                ps[:],
                                wst[wname][:, kt, i * P : (i + 1) * P],
                                tvec[:, kt : kt + 1],
                                start=(kt == 0),
                                stop=(kt == CT - 1),
                            )
                        nc.vector.tensor_scalar_add(
                            out=dst[:, i : i + 1], in0=ps[:],
                            scalar1=vec_sb[:, bidx, i : i + 1],
                        )
                for i in range(CT):
                    ps = pc.tile([P, 1], F32, name="cst", tag="cst")
                    for kt in range(CT):
                        nc.tensor.matmul(
                            ps[:],
                            wst["wp"][:, kt, i * P : (i + 1) * P],
                            cv[:, kt : kt + 1],
                            start=(kt == 0),
                            stop=(kt == CT - 1),
                        )
                    nc.vector.tensor_scalar_add(
                        out=cout[:, i : i + 1], in0=ps[:],
                        scalar1=vec_sb[:, 5, i : i + 1],
                    )

            # x_sb becomes the residual base: x + cout
            for i in range(CT):
                nc.gpsimd.tensor_scalar_add(
                    out=x_sb[i][:], in0=x_sb[i][:], scalar1=cout[:, i : i + 1]
                )

            # ------------------------------------------------------- q, k, v
            q8 = pp.tile([P, CT, N], FP8, name="q8", tag="q8")
            k8 = pp.tile([P, CT, N], FP8, name="k8", tag="k8")
            v8 = pp.tile([P, NT, C], FP8, name="v8", tag="v8")

            with tc.tile_pool(name="psum_qkv", bufs=4, space="PSUM") as pq:
                def qk_chunk(wname, dst, cst, j):
                    for i in range(CT):
                        ps = pq.tile([P, FD], F32, name="qk_ps", tag="qk_ps")
                        nc.tensor.matmul(
                            ps[:],
                            w8[wname][:, 0:CT, i * P : (i + 1) * P],
                            x8[:, 0:CT, j * FD : (j + 1) * FD],
                            start=True, stop=True, perf_mode=DR,
                        )
                        nc.vector.tensor_scalar(
                            out=dst[:, i, j * FD : (j + 1) * FD],
                            in0=ps[:],
                            scalar1=WSI, scalar2=cst[:, i : i + 1],
                            op0=ALU.mult, op1=ALU.add,
                        )

                def v_pair(t):
                    ps = pq.tile([P, 2, C], F32, name="v_ps", tag="v_ps")
                    for s2 in range(2):
                        nc.tensor.matmul(
                            ps[:, s2, :],
                            x8[:, 0:CT, (2 * t + s2) * P : (2 * t + s2 + 1) * P],
                            w8["wv"][:, 0:CT, :],
                            start=True, stop=True, perf_mode=DR,
                        )
                    nc.vector.tensor_scalar_mul(
                        out=v8[:, 2 * t : 2 * t + 2, :], in0=ps[:], scalar1=WSI
                    )

                # emission order keeps DVE ahead of the attention consumers:
                # scores(j=0) needs k chunks + q chunk 0; attn@V eats v pairs
                # progressively; later q chunks are only needed a chunk later.
                qk_chunk("wk", k8, ck, 0)
                qk_chunk("wk", k8, ck, 1)
                qk_chunk("wq", q8, cq, 0)
                for t in range(4):
                    v_pair(t)
                qk_chunk("wk", k8, ck, 2)
                qk_chunk("wk", k8, ck, 3)
                for t in range(4, MP):
                    v_pair(t)
                for j in range(1, NCH):
                    qk_chunk("wq", q8, cq, j)

            # ------------------------------------------------------ attention
            recip_sb = pp.tile([P, NCH, FD], F32, name="recipS", tag="recipS")

            with (
                tc.tile_pool(name="psum_ha", bufs=1, space="PSUM") as pha,
                tc.tile_pool(name="psum_s", bufs=1, space="PSUM") as psS,
                tc.tile_pool(name="psum_sc", bufs=2, space="PSUM") as psc,
                tc.tile_pool(name="psum_pr", bufs=1, space="PSUM") as ppr,
            ):
                pend = None  # (j, hA, S_ps) awaiting drain

                def drain(j, hA, S_ps):
                    # 1/S (shift/scale cancel), a8 = hA/S in fp8, out proj,
                    # epilogue: out = proj/WS + (x + cout), DMA out.
                    nc.vector.reciprocal_approx_fast(
                        out=recip_sb[:, j, :], in_=S_ps[:]
                    )
                    a8 = ap8.tile([P, CT, FD], FP8, name="a8", tag="a8")
                    for i in range(CT):
                        nc.vector.tensor_tensor(
                            a8[:, i, :], hA[i][:], recip_sb[:, j, :], op=ALU.mult
                        )
                    for i in range(CT):
                        ps = ppr.tile([P, FD], F32, name="pr", tag="pr")
                        nc.tensor.matmul(
                            ps[:],
                            w8["wp"][:, 0:CT, i * P : (i + 1) * P],
                            a8[:, 0:CT, :],
                            start=True, stop=True, perf_mode=DR,
                        )
                        tmp = scr.tile([P, FD], F32, name=f"tmp{i}", tag=f"tmp{i}")
                        nc.vector.tensor_scalar_mul(out=tmp[:], in0=ps[:], scalar1=WSI)
                        nc.gpsimd.tensor_tensor(
                            x_sb[i][:, j * FD : (j + 1) * FD],
                            tmp[:],
                            x_sb[i][:, j * FD : (j + 1) * FD],
                            op=ALU.add,
                        )
                        nc.sync.dma_start(
                            out=out_ext[i * P : (i + 1) * P, j * FD : (j + 1) * FD],
                            in_=x_sb[i][:, j * FD : (j + 1) * FD],
                        )

                for j in range(NCH):
                    hA = [
                        pha.tile([P, FD], F32, name=f"hA{i}", tag=f"hA{i}")
                        for i in range(CT)
                    ]
                    S_ps = psS.tile([P, FD], F32, name="S", tag="S")
                    for mp in range(MP):
                        sc = psc.tile([P, 2, FD], F32, name="sc", tag="sc")
                        for s2 in range(2):
                            m = 2 * mp + s2
                            nc.tensor.matmul(
                                sc[:, s2, :],
                                k8[:, 0:CT, m * P : (m + 1) * P],
                                q8[:, 0:CT, j * FD : (j + 1) * FD],
                                start=True, stop=True, perf_mode=DR,
                            )
                        e = ep.tile([P, 2, FD], FP8, name="e", tag="e")
                        nc.scalar.activation(
                            out=e[:], in_=sc[:], func=AF.Exp,
                            scale=SM_SCALE, bias=shift_ap[:],
                        )
                        if mp == 0 and pend is not None:
                            drain(*pend)
                            pend = None
                        for i in range(CT):
                            nc.tensor.matmul(
                                hA[i][:],
                                v8[:, 2 * mp : 2 * mp + 2, i * P : (i + 1) * P],
                                e[:, 0:2, :],
                                start=(mp == 0), stop=(mp == MP - 1),
                                perf_mode=DR,
                            )
                        nc.tensor.matmul(
                            S_ps[:],
                            ones8[:, 0:2, :],
                            e[:, 0:2, :],
                            start=(mp == 0), stop=(mp == MP - 1),
                            perf_mode=DR,
                        )
                    pend = (j, hA, S_ps)
                drain(*pend)
    return nc


_NC = None


def _get_nc():
    global _NC
    if _NC is None:
        _NC = build()
        _NC.finalize()
    return _NC


def _prepare_in_maps(inputs):
    x = np.ascontiguousarray(np.asarray(inputs["x"], dtype=np.float32))
    assert x.shape == (B, C, N), x.shape
    wqt = np.ascontiguousarray(np.asarray(inputs["Wq"], np.float32).T)
    wkt = np.ascontiguousarray(np.asarray(inputs["Wk"], np.float32).T)
    wvt = np.ascontiguousarray(np.asarray(inputs["Wv"], np.float32).T)
    wpt = np.ascontiguousarray(np.asarray(inputs["Wp"], np.float32).T)
    vecs = np.ascontiguousarray(
        np.stack(
            [
                np.asarray(inputs["gamma"], np.float32),
                np.asarray(inputs["beta"], np.float32),
                np.asarray(inputs["bq"], np.float32),
                np.asarray(inputs["bk"], np.float32),
                np.asarray(inputs["bv"], np.float32),
                np.asarray(inputs["bp"], np.float32),
            ]
        )
    )
    return [
        {
            "x": np.ascontiguousarray(x[b]),
            "wqt": wqt,
            "wkt": wkt,
            "wvt": wvt,
            "wpt": wpt,
            "vecs": vecs,
        }
        for b in range(B)
    ]


def kernel(**inputs):
    nc = _get_nc()
    in_maps = _prepare_in_maps(inputs)
    res = run_bass_kernel_spmd(nc, in_maps, list(range(B)))
    out = np.stack([np.asarray(res.results[b]["out"]) for b in range(B)])
    return out.astype(np.float32)
